# revision 3
# baseline (speedup 1.0000x reference)
"""Grouped MoE MLP (SwiGLU) kernel for Trainium2, 8 NeuronCores.

Strategy (expert-parallel, host-side routing):
  Tokens arrive pre-sorted by expert with per-expert counts.  The host
  cuts the expert-sorted token stream into 8 contiguous 2048-token
  chunks (one per core), so each core sees at most a couple of distinct
  experts.  Per core, the chunk's (expert, run) pieces are placed into a
  fixed slot structure (1024, 512, 256, 256); the per-core weight input
  holds only that core's distinct experts ([P, U, KH/KI, N] stacked on
  a U axis), and a tiny per-slot index tensor selects which weight
  entry each slot uses via dynamically-addressed DMA (bass.ds).  Every
  core runs the identical program: for each slot, a dense SwiGLU MLP of
  that slot's tokens with the indexed expert weights.  No device-side
  routing or collectives are needed.

  Layouts are transposed on the host so both GEMMs contract over the
  SBUF partition dimension with no on-chip transposes:
    GEMM1: out1^T[f, t] = sum_h W1[h, f] * x[t, h]   (h on partitions)
    SwiGLU on feature-partitioned tiles
    GEMM2: out^T[o, t]  = sum_f W2[f, o] * h[t, f]   (f on partitions)

  Weight DMAs ride the SP (sync) HWDGE ring; x / out / index DMAs ride
  the ACT (scalar) ring so the two streams don't FIFO-block each other.
  The device output is bf16 (accumulation stays fp32 in PSUM); the host
  casts back to fp32.
"""

import math
from contextlib import ExitStack

import ml_dtypes
import numpy as np

P = 128
HIDDEN = 2048
INTER = 1408
GU = 2 * INTER            # 2816 = gate+up columns
KH = HIDDEN // P          # 16 k-tiles for GEMM1
KI = INTER // P           # 11 k-tiles for GEMM2 / gate-up pair blocks
MO = HIDDEN // P          # 16 output feature blocks
N_CORES = 8
NT = 512                  # max tokens per chunk (matmul moving free dim)
MIXED_SLOTS = (1024, 512, 256, 256)   # per-core slot structure, 2048 rows
UNIFORM_SLOT = 768                    # fallback slot size

BF16 = ml_dtypes.bfloat16

_PROGRAM_CACHE: dict = {}


def _chunks(slot_rows: int, nt: int):
    out = []
    r = 0
    while r < slot_rows:
        c = min(nt, slot_rows - r)
        out.append((r, c))
        r += c
    return out


def _build_program(slot_sizes: tuple, n_entries: int, nt: int):
    import concourse.mybir as mybir
    import concourse.tile as tile
    from concourse import bacc, bass

    n_slots = len(slot_sizes)
    T = sum(slot_sizes)
    slot_off = np.concatenate([[0], np.cumsum(slot_sizes)]).astype(int)
    bf16 = mybir.dt.bfloat16
    f32 = mybir.dt.float32
    i32 = mybir.dt.int32

    nc = bacc.Bacc(None, target_bir_lowering=False, debug=False)
    xT = nc.dram_tensor("xT", [P, KH, T], bf16, kind="ExternalInput")
    w1 = nc.dram_tensor("w1", [P, n_entries, KH, GU], bf16, kind="ExternalInput")
    w2 = nc.dram_tensor("w2", [P, n_entries, KI, HIDDEN], bf16, kind="ExternalInput")
    widx = nc.dram_tensor("widx", [1, n_slots], i32, kind="ExternalInput")
    outT = nc.dram_tensor("outT", [P, MO, T], bf16, kind="ExternalOutput")

    with tile.TileContext(nc) as tc, ExitStack() as ctx:
        w1_pool = ctx.enter_context(tc.tile_pool(name="w1p", bufs=1))
        w2_pool = ctx.enter_context(tc.tile_pool(name="w2p", bufs=1))
        x_pool = ctx.enter_context(tc.tile_pool(name="xp", bufs=2))
        h_pool = ctx.enter_context(tc.tile_pool(name="hp", bufs=2))
        g_pool = ctx.enter_context(tc.tile_pool(name="gp", bufs=2))
        o_pool = ctx.enter_context(tc.tile_pool(name="op", bufs=4))
        i_pool = ctx.enter_context(tc.tile_pool(name="ip", bufs=1))
        ps1 = ctx.enter_context(tc.tile_pool(name="ps1", bufs=2, space="PSUM"))
        ps2 = ctx.enter_context(tc.tile_pool(name="ps2", bufs=2, space="PSUM"))

        # Per-slot weight-entry indices.  Slot 0 uses entry 0 by host-side
        # convention (entries are numbered in first-use order), so its
        # weight DMAs need no register dependency and start immediately.
        wvals = [None] * n_slots
        if n_entries > 1:
            widx_t = i_pool.tile([1, n_slots], i32)
            nc.scalar.dma_start(widx_t[:], widx[:])
            for s in range(1, n_slots):
                # skip_runtime_bounds_check: the device-side assert emitted
                # by the bounds check halts this runtime; indices are
                # host-generated and always within [0, n_entries).
                wvals[s] = nc.values_load(
                    widx_t[0:1, s : s + 1],
                    min_val=0,
                    max_val=n_entries - 1,
                    skip_runtime_bounds_check=True,
                )

        for s in range(n_slots):
            chunk_list = _chunks(slot_sizes[s], nt)
            # first x chunk of the slot goes ahead on the ACT ring; for the
            # very first slot it is split per k-tile so the first GEMM1
            # matmul only waits for ~128KB of x and ~720KB of w1
            c0_off, c0_n = chunk_list[0]
            xt0 = x_pool.tile([P, KH, c0_n], bf16, tag="xt")
            t00 = int(slot_off[s]) + c0_off
            if s == 0:
                for k in range(KH):
                    nc.scalar.dma_start(
                        xt0[:, k : k + 1, :], xT[:, k : k + 1, t00 : t00 + c0_n]
                    )
            else:
                nc.scalar.dma_start(xt0[:], xT[:, :, t00 : t00 + c0_n])
            w1t = w1_pool.tile([P, KH, GU], bf16)
            for k in range(KH):
                if wvals[s] is None:
                    src = w1[:, 0:1, k : k + 1, :]
                else:
                    src = w1[:, bass.ds(wvals[s], 1), k : k + 1, :]
                nc.sync.dma_start(w1t[:, k : k + 1, :], src)
            w2t = w2_pool.tile([P, KI, HIDDEN], bf16)
            for k in range(KI):
                if wvals[s] is None:
                    src = w2[:, 0:1, k : k + 1, :]
                else:
                    src = w2[:, bass.ds(wvals[s], 1), k : k + 1, :]
                nc.sync.dma_start(w2t[:, k : k + 1, :], src)
            for ci, (c_off, c_n) in enumerate(chunk_list):
                t0 = int(slot_off[s]) + c_off
                if ci == 0:
                    xt = xt0
                else:
                    xt = x_pool.tile([P, KH, c_n], bf16, tag="xt")
                    nc.scalar.dma_start(xt[:], xT[:, :, t0 : t0 + c_n])
                ht = h_pool.tile([P, KI, c_n], bf16, tag="ht")
                for mp in range(KI):
                    pg = ps1.tile([P, c_n], f32, tag="pg")
                    pu = ps1.tile([P, c_n], f32, tag="pu")
                    for k in range(KH):
                        nc.tensor.matmul(
                            pg[:],
                            w1t[:, k, mp * P : (mp + 1) * P],
                            xt[:, k, :],
                            start=(k == 0),
                            stop=(k == KH - 1),
                        )
                    for k in range(KH):
                        nc.tensor.matmul(
                            pu[:],
                            w1t[:, k, (KI + mp) * P : (KI + mp + 1) * P],
                            xt[:, k, :],
                            start=(k == 0),
                            stop=(k == KH - 1),
                        )
                    gt = g_pool.tile([P, c_n], bf16, tag="gt")
                    nc.scalar.activation(
                        gt[:], pg[:], mybir.ActivationFunctionType.Silu
                    )
                    nc.vector.tensor_mul(ht[:, mp, :], gt[:], pu[:])
                for m in range(MO):
                    po = ps2.tile([P, c_n], f32, tag="po")
                    for k in range(KI):
                        nc.tensor.matmul(
                            po[:],
                            w2t[:, k, m * P : (m + 1) * P],
                            ht[:, k, :],
                            start=(k == 0),
                            stop=(k == KI - 1),
                        )
                    om = o_pool.tile([P, c_n], bf16, tag="om")
                    nc.vector.tensor_copy(om[:], po[:])
                    nc.scalar.dma_start(outT[:, m, t0 : t0 + c_n], om[:])
    nc.compile()
    return nc


def _get_program(slot_sizes: tuple, n_entries: int, nt: int):
    key = (tuple(slot_sizes), n_entries, nt)
    if key not in _PROGRAM_CACHE:
        _PROGRAM_CACHE[key] = _build_program(tuple(slot_sizes), n_entries, nt)
    return _PROGRAM_CACHE[key]


def _pack_w1(w: np.ndarray) -> np.ndarray:
    # [HIDDEN, GU] f32 -> [P, KH, GU] bf16 with row h = 128*k + p
    return np.ascontiguousarray(
        w.reshape(KH, P, GU).transpose(1, 0, 2).astype(BF16)
    )


def _pack_w2(w: np.ndarray) -> np.ndarray:
    # [INTER, HIDDEN] f32 -> [P, KI, HIDDEN] bf16 with row f = 128*k + p
    return np.ascontiguousarray(
        w.reshape(KI, P, HIDDEN).transpose(1, 0, 2).astype(BF16)
    )


def _contig_cover(counts, slot_sizes, n_cores):
    """Cut the expert-sorted token stream into n_cores equal contiguous
    chunks and fit each chunk's (expert, run) pieces into the slot
    structure exactly.  Returns (core_shards, core_widx, n_entries) with
    core_shards[c][s] = (expert, expert_row0, nrows) per slot, or None
    if the counts don't fit this scheme."""
    total = sum(counts)
    tc = sum(slot_sizes)
    if total != n_cores * tc:
        return None

    # per-core contiguous pieces
    core_pieces = []
    e, used = 0, 0
    for _ in range(n_cores):
        need = tc
        pieces = []
        while need > 0:
            if e >= len(counts):
                return None
            avail = counts[e] - used
            take = min(avail, need)
            if take > 0:
                pieces.append([e, used, take])
                used += take
                need -= take
            if used >= counts[e]:
                e += 1
                used = 0
        core_pieces.append(pieces)

    gran = math.gcd(*slot_sizes)
    core_shards = []
    core_widx = []
    n_entries = 1
    for pieces in core_pieces:
        if any(p[2] % gran for p in pieces):
            return None
        # exact-fit pieces (largest first) onto slots (largest first)
        order = sorted(range(len(pieces)), key=lambda i: -pieces[i][2])
        avail = sorted(range(len(slot_sizes)), key=lambda s: -slot_sizes[s])
        slot_assign = [None] * len(slot_sizes)
        for pi in order:
            e_, r0, n = pieces[pi]
            rem = n
            r = r0
            for s in list(avail):
                sz = slot_sizes[s]
                if sz <= rem:
                    slot_assign[s] = (e_, r, sz)
                    r += sz
                    rem -= sz
                    avail.remove(s)
            if rem:
                return None
        if avail:
            return None
        # entries in first-use (slot) order; slot 0 always entry 0
        entries = []
        widx = []
        for s in range(len(slot_sizes)):
            e_ = slot_assign[s][0]
            if e_ not in entries:
                entries.append(e_)
            widx.append(entries.index(e_))
        core_shards.append(slot_assign)
        core_widx.append((widx, entries))
        n_entries = max(n_entries, len(entries))
    return core_shards, core_widx, n_entries


def _mixed_cover(counts, slot_sizes):
    """Exact-cover counts by pieces {size: N_CORES per size}. Returns
    per-core shard lists [(expert, row0, nrows), ...] ordered like
    slot_sizes, or None if no exact cover exists."""
    from collections import Counter

    sizes_desc = sorted(slot_sizes, reverse=True)
    avail = Counter(slot_sizes)
    for s in avail:
        avail[s] *= N_CORES
    per_expert: list = [None] * len(counts)

    def cover(rem, max_size):
        if rem == 0:
            return []
        for s in sorted(set(avail), reverse=True):
            if s > max_size or s > rem or avail[s] == 0:
                continue
            avail[s] -= 1
            sub = cover(rem - s, s)
            if sub is not None:
                return [s] + sub
            avail[s] += 1
        return None

    # Largest counts first so big pieces go where they must.
    order = sorted(range(len(counts)), key=lambda e: -counts[e])
    for e in order:
        pieces = cover(counts[e], max(sizes_desc))
        if pieces is None:
            return None
        per_expert[e] = pieces

    # Build shard pieces and deal them out per size class.
    by_size: dict = {s: [] for s in set(slot_sizes)}
    for e in range(len(counts)):
        r = 0
        for s in sorted(per_expert[e], reverse=True):
            by_size[s].append((e, r, s))
            r += s
    # Pad classes with empty shards (possible when sum(counts) is short).
    for s, lst in by_size.items():
        want = slot_sizes.count(s) * N_CORES
        while len(lst) < want:
            lst.append((0, 0, 0))
        if len(lst) != want:
            return None

    cores = []
    for r in range(N_CORES):
        shards = []
        used = {s: 0 for s in by_size}
        for s in slot_sizes:
            shards.append(by_size[s][r * slot_sizes.count(s) + used[s]])
            used[s] += 1
        cores.append(shards)
    return cores


def _uniform_cover(counts, slot):
    shards = []
    for e in range(len(counts)):
        r = 0
        while r < counts[e]:
            n = min(slot, counts[e] - r)
            shards.append((e, r, n))
            r += n
    n_slots = max(1, math.ceil(len(shards) / N_CORES))
    while len(shards) < N_CORES * n_slots:
        shards.append((0, 0, 0))
    return [shards[r * n_slots : (r + 1) * n_slots] for r in range(N_CORES)], n_slots


def _run(
    hidden_states: np.ndarray,
    merged_gate_up_proj: np.ndarray,
    merged_down_proj: np.ndarray,
    num_tokens_per_expert: np.ndarray,
    trace: bool = False,
):
    from concourse.bass_utils import run_bass_kernel_spmd

    counts = [int(c) for c in np.asarray(num_tokens_per_expert)]
    n_experts = len(counts)
    offs = np.concatenate([[0], np.cumsum(counts)]).astype(int)
    total = int(offs[-1])

    contig = _contig_cover(counts, MIXED_SLOTS, N_CORES)
    if contig is not None:
        core_shards, core_widx, n_entries = contig
        slot_sizes = MIXED_SLOTS
    else:
        core_shards = _mixed_cover(counts, MIXED_SLOTS)
        if core_shards is not None:
            slot_sizes = MIXED_SLOTS
        else:
            core_shards, n_slots = _uniform_cover(counts, UNIFORM_SLOT)
            slot_sizes = (UNIFORM_SLOT,) * n_slots
        # generic fallback: one weight entry per slot
        n_entries = len(slot_sizes)
        core_widx = [
            (list(range(len(slot_sizes))), [e for (e, _, _) in shards])
            for shards in core_shards
        ]

    slot_off = np.concatenate([[0], np.cumsum(slot_sizes)]).astype(int)
    T = int(slot_off[-1])

    nc = _get_program(slot_sizes, n_entries, NT)

    w1_packed = {}
    w2_packed = {}
    for shards, (widx, entries) in zip(core_shards, core_widx):
        for e in entries:
            if e not in w1_packed:
                w1_packed[e] = _pack_w1(merged_gate_up_proj[e])
                w2_packed[e] = _pack_w2(merged_down_proj[e])
    zeros_w1 = None
    zeros_w2 = None
    x_bf16 = hidden_states.astype(BF16)

    in_maps = []
    for r in range(N_CORES):
        shards = core_shards[r]
        widx, entries = core_widx[r]
        x_core = np.zeros((T, HIDDEN), dtype=BF16)
        for s, (e, r0, n) in enumerate(shards):
            if n:
                x_core[slot_off[s] : slot_off[s] + n] = x_bf16[
                    offs[e] + r0 : offs[e] + r0 + n
                ]
        # [T, HIDDEN] -> [P, KH, T] with column h = 128*k + p
        xT_core = np.ascontiguousarray(
            x_core.T.reshape(KH, P, T).transpose(1, 0, 2)
        )
        w1_list = [w1_packed[e] for e in entries]
        w2_list = [w2_packed[e] for e in entries]
        while len(w1_list) < n_entries:
            if zeros_w1 is None:
                zeros_w1 = np.zeros_like(next(iter(w1_packed.values())))
                zeros_w2 = np.zeros_like(next(iter(w2_packed.values())))
            w1_list.append(zeros_w1)
            w2_list.append(zeros_w2)
        in_maps.append(
            {
                "xT": xT_core,
                "w1": np.ascontiguousarray(np.stack(w1_list, axis=1)),
                "w2": np.ascontiguousarray(np.stack(w2_list, axis=1)),
                "widx": np.asarray([widx], dtype=np.int32),
            }
        )

    res = run_bass_kernel_spmd(nc, in_maps, list(range(N_CORES)), trace=trace)

    out = np.empty((total, HIDDEN), dtype=np.float32)
    for r in range(N_CORES):
        # [P, MO, T] -> [T, HIDDEN] with column o = 128*m + p
        o_core = res.results[r]["outT"].transpose(2, 1, 0).reshape(T, HIDDEN)
        for s, (e, r0, n) in enumerate(core_shards[r]):
            if n:
                out[offs[e] + r0 : offs[e] + r0 + n] = o_core[
                    slot_off[s] : slot_off[s] + n
                ]
    return out, res


def kernel(**inputs) -> np.ndarray:
    return _run(**inputs, trace=False)[0]


def run_traced(**inputs):
    return _run(**inputs, trace=True)


# revision 5
# speedup vs baseline: 1.0493x; 1.0493x over previous
"""Grouped MoE MLP (SwiGLU) kernel for Trainium2, 8 NeuronCores.

Strategy (expert-parallel, host-side routing):
  Tokens arrive pre-sorted by expert with per-expert counts.  The host
  cuts the expert-sorted token stream into 8 contiguous 2048-token
  chunks (one per core), so each core sees at most a couple of distinct
  experts.  Per core, the chunk's (expert, run) pieces are placed into a
  fixed slot structure (1024, 512, 256, 256); the per-core weight input
  holds only that core's distinct experts ([P, U, KH/KI, N] stacked on
  a U axis), and a tiny per-slot index tensor selects which weight
  entry each slot uses via dynamically-addressed DMA (bass.ds).  Every
  core runs the identical program: for each slot, a dense SwiGLU MLP of
  that slot's tokens with the indexed expert weights.  No device-side
  routing or collectives are needed.

  Layouts are transposed on the host so both GEMMs contract over the
  SBUF partition dimension with no on-chip transposes:
    GEMM1: out1^T[f, t] = sum_h W1[h, f] * x[t, h]   (h on partitions)
    SwiGLU on feature-partitioned tiles
    GEMM2: out^T[o, t]  = sum_f W2[f, o] * h[t, f]   (f on partitions)

  Weight DMAs ride the GpSimd (SWDGE) ring — the Pool engine is
  otherwise idle so weight streaming never queues behind compute
  semaphores; x / out / index DMAs ride the SP (sync) HWDGE ring.
  The device output is bf16 (accumulation stays fp32 in PSUM); the host
  casts back to fp32.
"""

import math
from contextlib import ExitStack

import ml_dtypes
import numpy as np

P = 128
HIDDEN = 2048
INTER = 1408
GU = 2 * INTER            # 2816 = gate+up columns
KH = HIDDEN // P          # 16 k-tiles for GEMM1
KI = INTER // P           # 11 k-tiles for GEMM2 / gate-up pair blocks
MO = HIDDEN // P          # 16 output feature blocks
N_CORES = 8
NT = 512                  # max tokens per chunk (matmul moving free dim)
MIXED_SLOTS = (1024, 512, 256, 256)   # per-core slot structure, 2048 rows
UNIFORM_SLOT = 768                    # fallback slot size

BF16 = ml_dtypes.bfloat16

_PROGRAM_CACHE: dict = {}


def _chunks(slot_rows: int, nt: int):
    out = []
    r = 0
    while r < slot_rows:
        c = min(nt, slot_rows - r)
        out.append((r, c))
        r += c
    return out


def _build_program(slot_sizes: tuple, n_entries: int, nt: int):
    import concourse.mybir as mybir
    import concourse.tile as tile
    from concourse import bacc, bass

    n_slots = len(slot_sizes)
    T = sum(slot_sizes)
    slot_off = np.concatenate([[0], np.cumsum(slot_sizes)]).astype(int)
    bf16 = mybir.dt.bfloat16
    f32 = mybir.dt.float32
    i32 = mybir.dt.int32

    nc = bacc.Bacc(None, target_bir_lowering=False, debug=False)
    xT = nc.dram_tensor("xT", [P, KH, T], bf16, kind="ExternalInput")
    w1 = nc.dram_tensor("w1", [P, n_entries, KH, GU], bf16, kind="ExternalInput")
    w2 = nc.dram_tensor("w2", [P, n_entries, KI, HIDDEN], bf16, kind="ExternalInput")
    widx = nc.dram_tensor("widx", [1, n_slots], i32, kind="ExternalInput")
    outT = nc.dram_tensor("outT", [P, MO, T], bf16, kind="ExternalOutput")

    with tile.TileContext(nc) as tc, ExitStack() as ctx:
        w1_pool = ctx.enter_context(tc.tile_pool(name="w1p", bufs=1))
        w2_pool = ctx.enter_context(tc.tile_pool(name="w2p", bufs=1))
        x_pool = ctx.enter_context(tc.tile_pool(name="xp", bufs=2))
        h_pool = ctx.enter_context(tc.tile_pool(name="hp", bufs=2))
        g_pool = ctx.enter_context(tc.tile_pool(name="gp", bufs=2))
        o_pool = ctx.enter_context(tc.tile_pool(name="op", bufs=4))
        i_pool = ctx.enter_context(tc.tile_pool(name="ip", bufs=1))
        ps1 = ctx.enter_context(tc.tile_pool(name="ps1", bufs=2, space="PSUM"))
        ps2 = ctx.enter_context(tc.tile_pool(name="ps2", bufs=2, space="PSUM"))

        # Per-slot weight-entry indices.  Slot 0 uses entry 0 by host-side
        # convention (entries are numbered in first-use order), so its
        # weight DMAs need no register dependency and start immediately.
        wvals = [None] * n_slots
        if n_entries > 1:
            widx_t = i_pool.tile([1, n_slots], i32)
            nc.sync.dma_start(widx_t[:], widx[:])
            for s in range(1, n_slots):
                # skip_runtime_bounds_check: the device-side assert emitted
                # by the bounds check halts this runtime; indices are
                # host-generated and always within [0, n_entries).
                wvals[s] = nc.values_load(
                    widx_t[0:1, s : s + 1],
                    min_val=0,
                    max_val=n_entries - 1,
                    skip_runtime_bounds_check=True,
                )

        for s in range(n_slots):
            chunk_list = _chunks(slot_sizes[s], nt)
            # first x chunk of the slot goes ahead on the ACT ring; for the
            # very first slot it is split per k-tile so the first GEMM1
            # matmul only waits for ~128KB of x and ~720KB of w1
            c0_off, c0_n = chunk_list[0]
            xt0 = x_pool.tile([P, KH, c0_n], bf16, tag="xt")
            t00 = int(slot_off[s]) + c0_off
            if s == 0:
                for k in range(KH):
                    nc.sync.dma_start(
                        xt0[:, k : k + 1, :], xT[:, k : k + 1, t00 : t00 + c0_n]
                    )
            else:
                nc.sync.dma_start(xt0[:], xT[:, :, t00 : t00 + c0_n])
            w1t = w1_pool.tile([P, KH, GU], bf16)
            for k in range(KH):
                if wvals[s] is None:
                    src = w1[:, 0:1, k : k + 1, :]
                else:
                    src = w1[:, bass.ds(wvals[s], 1), k : k + 1, :]
                nc.gpsimd.dma_start(w1t[:, k : k + 1, :], src)
            w2t = w2_pool.tile([P, KI, HIDDEN], bf16)
            for k in range(KI):
                if wvals[s] is None:
                    src = w2[:, 0:1, k : k + 1, :]
                else:
                    src = w2[:, bass.ds(wvals[s], 1), k : k + 1, :]
                nc.gpsimd.dma_start(w2t[:, k : k + 1, :], src)
            for ci, (c_off, c_n) in enumerate(chunk_list):
                t0 = int(slot_off[s]) + c_off
                if ci == 0:
                    xt = xt0
                else:
                    xt = x_pool.tile([P, KH, c_n], bf16, tag="xt")
                    nc.sync.dma_start(xt[:], xT[:, :, t0 : t0 + c_n])
                ht = h_pool.tile([P, KI, c_n], bf16, tag="ht")
                for mp in range(KI):
                    pg = ps1.tile([P, c_n], f32, tag="pg")
                    pu = ps1.tile([P, c_n], f32, tag="pu")
                    for k in range(KH):
                        nc.tensor.matmul(
                            pg[:],
                            w1t[:, k, mp * P : (mp + 1) * P],
                            xt[:, k, :],
                            start=(k == 0),
                            stop=(k == KH - 1),
                        )
                    for k in range(KH):
                        nc.tensor.matmul(
                            pu[:],
                            w1t[:, k, (KI + mp) * P : (KI + mp + 1) * P],
                            xt[:, k, :],
                            start=(k == 0),
                            stop=(k == KH - 1),
                        )
                    gt = g_pool.tile([P, c_n], bf16, tag="gt")
                    nc.scalar.activation(
                        gt[:], pg[:], mybir.ActivationFunctionType.Silu
                    )
                    nc.vector.tensor_mul(ht[:, mp, :], gt[:], pu[:])
                for m in range(MO):
                    po = ps2.tile([P, c_n], f32, tag="po")
                    for k in range(KI):
                        nc.tensor.matmul(
                            po[:],
                            w2t[:, k, m * P : (m + 1) * P],
                            ht[:, k, :],
                            start=(k == 0),
                            stop=(k == KI - 1),
                        )
                    om = o_pool.tile([P, c_n], bf16, tag="om")
                    nc.vector.tensor_copy(om[:], po[:])
                    nc.sync.dma_start(outT[:, m, t0 : t0 + c_n], om[:])
    nc.compile()
    return nc


def _get_program(slot_sizes: tuple, n_entries: int, nt: int):
    key = (tuple(slot_sizes), n_entries, nt)
    if key not in _PROGRAM_CACHE:
        _PROGRAM_CACHE[key] = _build_program(tuple(slot_sizes), n_entries, nt)
    return _PROGRAM_CACHE[key]


def _pack_w1(w: np.ndarray) -> np.ndarray:
    # [HIDDEN, GU] f32 -> [P, KH, GU] bf16 with row h = 128*k + p
    return np.ascontiguousarray(
        w.reshape(KH, P, GU).transpose(1, 0, 2).astype(BF16)
    )


def _pack_w2(w: np.ndarray) -> np.ndarray:
    # [INTER, HIDDEN] f32 -> [P, KI, HIDDEN] bf16 with row f = 128*k + p
    return np.ascontiguousarray(
        w.reshape(KI, P, HIDDEN).transpose(1, 0, 2).astype(BF16)
    )


def _contig_cover(counts, slot_sizes, n_cores):
    """Cut the expert-sorted token stream into n_cores equal contiguous
    chunks and fit each chunk's (expert, run) pieces into the slot
    structure exactly.  Returns (core_shards, core_widx, n_entries) with
    core_shards[c][s] = (expert, expert_row0, nrows) per slot, or None
    if the counts don't fit this scheme."""
    total = sum(counts)
    tc = sum(slot_sizes)
    if total != n_cores * tc:
        return None

    # per-core contiguous pieces
    core_pieces = []
    e, used = 0, 0
    for _ in range(n_cores):
        need = tc
        pieces = []
        while need > 0:
            if e >= len(counts):
                return None
            avail = counts[e] - used
            take = min(avail, need)
            if take > 0:
                pieces.append([e, used, take])
                used += take
                need -= take
            if used >= counts[e]:
                e += 1
                used = 0
        core_pieces.append(pieces)

    gran = math.gcd(*slot_sizes)
    core_shards = []
    core_widx = []
    n_entries = 1
    for pieces in core_pieces:
        if any(p[2] % gran for p in pieces):
            return None
        # exact-fit pieces (largest first) onto slots (largest first)
        order = sorted(range(len(pieces)), key=lambda i: -pieces[i][2])
        avail = sorted(range(len(slot_sizes)), key=lambda s: -slot_sizes[s])
        slot_assign = [None] * len(slot_sizes)
        for pi in order:
            e_, r0, n = pieces[pi]
            rem = n
            r = r0
            for s in list(avail):
                sz = slot_sizes[s]
                if sz <= rem:
                    slot_assign[s] = (e_, r, sz)
                    r += sz
                    rem -= sz
                    avail.remove(s)
            if rem:
                return None
        if avail:
            return None
        # entries in first-use (slot) order; slot 0 always entry 0
        entries = []
        widx = []
        for s in range(len(slot_sizes)):
            e_ = slot_assign[s][0]
            if e_ not in entries:
                entries.append(e_)
            widx.append(entries.index(e_))
        core_shards.append(slot_assign)
        core_widx.append((widx, entries))
        n_entries = max(n_entries, len(entries))
    return core_shards, core_widx, n_entries


def _mixed_cover(counts, slot_sizes):
    """Exact-cover counts by pieces {size: N_CORES per size}. Returns
    per-core shard lists [(expert, row0, nrows), ...] ordered like
    slot_sizes, or None if no exact cover exists."""
    from collections import Counter

    sizes_desc = sorted(slot_sizes, reverse=True)
    avail = Counter(slot_sizes)
    for s in avail:
        avail[s] *= N_CORES
    per_expert: list = [None] * len(counts)

    def cover(rem, max_size):
        if rem == 0:
            return []
        for s in sorted(set(avail), reverse=True):
            if s > max_size or s > rem or avail[s] == 0:
                continue
            avail[s] -= 1
            sub = cover(rem - s, s)
            if sub is not None:
                return [s] + sub
            avail[s] += 1
        return None

    # Largest counts first so big pieces go where they must.
    order = sorted(range(len(counts)), key=lambda e: -counts[e])
    for e in order:
        pieces = cover(counts[e], max(sizes_desc))
        if pieces is None:
            return None
        per_expert[e] = pieces

    # Build shard pieces and deal them out per size class.
    by_size: dict = {s: [] for s in set(slot_sizes)}
    for e in range(len(counts)):
        r = 0
        for s in sorted(per_expert[e], reverse=True):
            by_size[s].append((e, r, s))
            r += s
    # Pad classes with empty shards (possible when sum(counts) is short).
    for s, lst in by_size.items():
        want = slot_sizes.count(s) * N_CORES
        while len(lst) < want:
            lst.append((0, 0, 0))
        if len(lst) != want:
            return None

    cores = []
    for r in range(N_CORES):
        shards = []
        used = {s: 0 for s in by_size}
        for s in slot_sizes:
            shards.append(by_size[s][r * slot_sizes.count(s) + used[s]])
            used[s] += 1
        cores.append(shards)
    return cores


def _uniform_cover(counts, slot):
    shards = []
    for e in range(len(counts)):
        r = 0
        while r < counts[e]:
            n = min(slot, counts[e] - r)
            shards.append((e, r, n))
            r += n
    n_slots = max(1, math.ceil(len(shards) / N_CORES))
    while len(shards) < N_CORES * n_slots:
        shards.append((0, 0, 0))
    return [shards[r * n_slots : (r + 1) * n_slots] for r in range(N_CORES)], n_slots


def _run(
    hidden_states: np.ndarray,
    merged_gate_up_proj: np.ndarray,
    merged_down_proj: np.ndarray,
    num_tokens_per_expert: np.ndarray,
    trace: bool = False,
):
    from concourse.bass_utils import run_bass_kernel_spmd

    counts = [int(c) for c in np.asarray(num_tokens_per_expert)]
    n_experts = len(counts)
    offs = np.concatenate([[0], np.cumsum(counts)]).astype(int)
    total = int(offs[-1])

    contig = _contig_cover(counts, MIXED_SLOTS, N_CORES)
    if contig is not None:
        core_shards, core_widx, n_entries = contig
        slot_sizes = MIXED_SLOTS
    else:
        core_shards = _mixed_cover(counts, MIXED_SLOTS)
        if core_shards is not None:
            slot_sizes = MIXED_SLOTS
        else:
            core_shards, n_slots = _uniform_cover(counts, UNIFORM_SLOT)
            slot_sizes = (UNIFORM_SLOT,) * n_slots
        # generic fallback: one weight entry per slot
        n_entries = len(slot_sizes)
        core_widx = [
            (list(range(len(slot_sizes))), [e for (e, _, _) in shards])
            for shards in core_shards
        ]

    slot_off = np.concatenate([[0], np.cumsum(slot_sizes)]).astype(int)
    T = int(slot_off[-1])

    nc = _get_program(slot_sizes, n_entries, NT)

    w1_packed = {}
    w2_packed = {}
    for shards, (widx, entries) in zip(core_shards, core_widx):
        for e in entries:
            if e not in w1_packed:
                w1_packed[e] = _pack_w1(merged_gate_up_proj[e])
                w2_packed[e] = _pack_w2(merged_down_proj[e])
    zeros_w1 = None
    zeros_w2 = None
    x_bf16 = hidden_states.astype(BF16)

    in_maps = []
    for r in range(N_CORES):
        shards = core_shards[r]
        widx, entries = core_widx[r]
        x_core = np.zeros((T, HIDDEN), dtype=BF16)
        for s, (e, r0, n) in enumerate(shards):
            if n:
                x_core[slot_off[s] : slot_off[s] + n] = x_bf16[
                    offs[e] + r0 : offs[e] + r0 + n
                ]
        # [T, HIDDEN] -> [P, KH, T] with column h = 128*k + p
        xT_core = np.ascontiguousarray(
            x_core.T.reshape(KH, P, T).transpose(1, 0, 2)
        )
        w1_list = [w1_packed[e] for e in entries]
        w2_list = [w2_packed[e] for e in entries]
        while len(w1_list) < n_entries:
            if zeros_w1 is None:
                zeros_w1 = np.zeros_like(next(iter(w1_packed.values())))
                zeros_w2 = np.zeros_like(next(iter(w2_packed.values())))
            w1_list.append(zeros_w1)
            w2_list.append(zeros_w2)
        in_maps.append(
            {
                "xT": xT_core,
                "w1": np.ascontiguousarray(np.stack(w1_list, axis=1)),
                "w2": np.ascontiguousarray(np.stack(w2_list, axis=1)),
                "widx": np.asarray([widx], dtype=np.int32),
            }
        )

    res = run_bass_kernel_spmd(nc, in_maps, list(range(N_CORES)), trace=trace)

    out = np.empty((total, HIDDEN), dtype=np.float32)
    for r in range(N_CORES):
        # [P, MO, T] -> [T, HIDDEN] with column o = 128*m + p
        o_core = res.results[r]["outT"].transpose(2, 1, 0).reshape(T, HIDDEN)
        for s, (e, r0, n) in enumerate(core_shards[r]):
            if n:
                out[offs[e] + r0 : offs[e] + r0 + n] = o_core[
                    slot_off[s] : slot_off[s] + n
                ]
    return out, res


def kernel(**inputs) -> np.ndarray:
    return _run(**inputs, trace=False)[0]


def run_traced(**inputs):
    return _run(**inputs, trace=True)


# revision 7
# speedup vs baseline: 1.0979x; 1.0463x over previous
"""Grouped MoE MLP (SwiGLU) kernel for Trainium2, 8 NeuronCores.

Strategy (expert-parallel, host-side routing):
  Tokens arrive pre-sorted by expert with per-expert counts.  The host
  cuts the expert-sorted token stream into 8 contiguous 2048-token
  chunks (one per core), so each core sees at most a couple of distinct
  experts.  Per core, the chunk's (expert, run) pieces are placed into a
  fixed slot structure (1024, 512, 256, 256); the per-core weight input
  holds only that core's distinct experts ([P, U, KH/KI, N] stacked on
  a U axis), and a tiny per-slot index tensor selects which weight
  entry each slot uses via dynamically-addressed DMA (bass.ds).  Every
  core runs the identical program: for each slot, a dense SwiGLU MLP of
  that slot's tokens with the indexed expert weights.  No device-side
  routing or collectives are needed.

  Layouts are transposed on the host so both GEMMs contract over the
  SBUF partition dimension with no on-chip transposes:
    GEMM1: out1^T[f, t] = sum_h W1[h, f] * x[t, h]   (h on partitions)
    SwiGLU on feature-partitioned tiles
    GEMM2: out^T[o, t]  = sum_f W2[f, o] * h[t, f]   (f on partitions)

  Weight DMAs ride the GpSimd (SWDGE) ring — the Pool engine is
  otherwise idle so weight streaming never queues behind compute
  semaphores; x / out / index DMAs ride the SP (sync) HWDGE ring.
  The device output is bf16 (accumulation stays fp32 in PSUM); the host
  casts back to fp32.
"""

import math
from contextlib import ExitStack

import ml_dtypes
import numpy as np

P = 128
HIDDEN = 2048
INTER = 1408
GU = 2 * INTER            # 2816 = gate+up columns
KH = HIDDEN // P          # 16 k-tiles for GEMM1
KI = INTER // P           # 11 k-tiles for GEMM2 / gate-up pair blocks
MO = HIDDEN // P          # 16 output feature blocks
N_CORES = 8
NT = 512                  # max tokens per chunk (matmul moving free dim)
MIXED_SLOTS = (1024, 512, 256, 256)   # per-core slot structure, 2048 rows
UNIFORM_SLOT = 768                    # fallback slot size

BF16 = ml_dtypes.bfloat16

_PROGRAM_CACHE: dict = {}


def _chunks(slot_rows: int, nt: int):
    out = []
    r = 0
    while r < slot_rows:
        c = min(nt, slot_rows - r)
        out.append((r, c))
        r += c
    return out


def _build_program(slot_sizes: tuple, n_entries: int, nt: int):
    import concourse.mybir as mybir
    import concourse.tile as tile
    from concourse import bacc, bass

    n_slots = len(slot_sizes)
    T = sum(slot_sizes)
    slot_off = np.concatenate([[0], np.cumsum(slot_sizes)]).astype(int)
    bf16 = mybir.dt.bfloat16
    f32 = mybir.dt.float32
    i32 = mybir.dt.int32

    nc = bacc.Bacc(None, target_bir_lowering=False, debug=False)
    xT = nc.dram_tensor("xT", [P, KH, T], bf16, kind="ExternalInput")
    # weights stored block-major so they stream in ~1.1MB / ~0.36MB pieces:
    #   w1[p, u, mp, k, 0:128]  = gate cols of output block mp
    #   w1[p, u, mp, k, 128:256] = up  cols of output block mp
    #   w2[p, u, m, k, 0:128]   = cols of output block m
    w1 = nc.dram_tensor(
        "w1", [P, n_entries, KI, KH, 2 * P], bf16, kind="ExternalInput"
    )
    w2 = nc.dram_tensor(
        "w2", [P, n_entries, MO, KI, P], bf16, kind="ExternalInput"
    )
    widx = nc.dram_tensor("widx", [1, n_slots], i32, kind="ExternalInput")
    outT = nc.dram_tensor("outT", [P, MO, T], bf16, kind="ExternalOutput")

    with tile.TileContext(nc) as tc, ExitStack() as ctx:
        w1_pool = ctx.enter_context(tc.tile_pool(name="w1p", bufs=4))
        w2_pool = ctx.enter_context(tc.tile_pool(name="w2p", bufs=4))
        x_pool = ctx.enter_context(tc.tile_pool(name="xp", bufs=2))
        h_pool = ctx.enter_context(tc.tile_pool(name="hp", bufs=2))
        g_pool = ctx.enter_context(tc.tile_pool(name="gp", bufs=2))
        o_pool = ctx.enter_context(tc.tile_pool(name="op", bufs=4))
        i_pool = ctx.enter_context(tc.tile_pool(name="ip", bufs=1))
        ps1 = ctx.enter_context(tc.tile_pool(name="ps1", bufs=2, space="PSUM"))
        ps2 = ctx.enter_context(tc.tile_pool(name="ps2", bufs=2, space="PSUM"))

        # Per-slot weight-entry indices.  Slot 0 uses entry 0 by host-side
        # convention (entries are numbered in first-use order), so its
        # weight DMAs need no register dependency and start immediately.
        wvals = [None] * n_slots
        if n_entries > 1:
            widx_t = i_pool.tile([1, n_slots], i32)
            nc.sync.dma_start(widx_t[:], widx[:])
            for s in range(1, n_slots):
                # skip_runtime_bounds_check: the device-side assert emitted
                # by the bounds check halts this runtime; indices are
                # host-generated and always within [0, n_entries).
                wvals[s] = nc.values_load(
                    widx_t[0:1, s : s + 1],
                    min_val=0,
                    max_val=n_entries - 1,
                    skip_runtime_bounds_check=True,
                )

        for s in range(n_slots):
            sz = slot_sizes[s]
            chunk_list = _chunks(sz, nt)
            t_base = int(slot_off[s])
            # whole-slot x resident in SBUF; first slot split per k-tile so
            # the first GEMM1 matmul only waits for ~256KB of x + one block
            xt = x_pool.tile([P, KH, sz], bf16, tag="xt")
            if s == 0:
                for k in range(KH):
                    nc.sync.dma_start(
                        xt[:, k : k + 1, :], xT[:, k : k + 1, t_base : t_base + sz]
                    )
            else:
                nc.sync.dma_start(xt[:], xT[:, :, t_base : t_base + sz])
            ht = h_pool.tile([P, KI, sz], bf16, tag="ht")
            for mp in range(KI):
                w1b = w1_pool.tile([P, 1, KH, 2 * P], bf16, tag="w1b")
                if wvals[s] is None:
                    nc.gpsimd.dma_start(w1b[:], w1[:, 0:1, mp, :, :])
                else:
                    nc.gpsimd.dma_start(
                        w1b[:], w1[:, bass.ds(wvals[s], 1), mp, :, :]
                    )
                for c_off, c_n in chunk_list:
                    pg = ps1.tile([P, c_n], f32, tag="pg")
                    pu = ps1.tile([P, c_n], f32, tag="pu")
                    for k in range(KH):
                        nc.tensor.matmul(
                            pg[:],
                            w1b[:, 0, k, 0:P],
                            xt[:, k, c_off : c_off + c_n],
                            start=(k == 0),
                            stop=(k == KH - 1),
                        )
                    for k in range(KH):
                        nc.tensor.matmul(
                            pu[:],
                            w1b[:, 0, k, P : 2 * P],
                            xt[:, k, c_off : c_off + c_n],
                            start=(k == 0),
                            stop=(k == KH - 1),
                        )
                    gt = g_pool.tile([P, c_n], bf16, tag="gt")
                    nc.scalar.activation(
                        gt[:], pg[:], mybir.ActivationFunctionType.Silu
                    )
                    nc.vector.tensor_mul(ht[:, mp, c_off : c_off + c_n], gt[:], pu[:])
            for m in range(MO):
                w2b = w2_pool.tile([P, 1, KI, P], bf16, tag="w2b")
                if wvals[s] is None:
                    nc.gpsimd.dma_start(w2b[:], w2[:, 0:1, m, :, :])
                else:
                    nc.gpsimd.dma_start(
                        w2b[:], w2[:, bass.ds(wvals[s], 1), m, :, :]
                    )
                for c_off, c_n in chunk_list:
                    po = ps2.tile([P, c_n], f32, tag="po")
                    for k in range(KI):
                        nc.tensor.matmul(
                            po[:],
                            w2b[:, 0, k, :],
                            ht[:, k, c_off : c_off + c_n],
                            start=(k == 0),
                            stop=(k == KI - 1),
                        )
                    om = o_pool.tile([P, c_n], bf16, tag="om")
                    nc.vector.tensor_copy(om[:], po[:])
                    nc.sync.dma_start(
                        outT[:, m, t_base + c_off : t_base + c_off + c_n], om[:]
                    )
    nc.compile()
    return nc


def _get_program(slot_sizes: tuple, n_entries: int, nt: int):
    key = (tuple(slot_sizes), n_entries, nt)
    if key not in _PROGRAM_CACHE:
        _PROGRAM_CACHE[key] = _build_program(tuple(slot_sizes), n_entries, nt)
    return _PROGRAM_CACHE[key]


def _pack_w1(w: np.ndarray) -> np.ndarray:
    # [HIDDEN, GU] f32 -> [P, KI, KH, 2P] bf16, block mp holding gate cols
    # [mp*128:(mp+1)*128] then up cols [(KI+mp)*128:(KI+mp+1)*128];
    # row h = 128*k + p
    wk = w.reshape(KH, P, GU).astype(BF16)          # [k, p, col]
    gate = wk[:, :, : KI * P].reshape(KH, P, KI, P)
    up = wk[:, :, KI * P :].reshape(KH, P, KI, P)
    blk = np.concatenate([gate, up], axis=3)        # [k, p, mp, 2P]
    return np.ascontiguousarray(blk.transpose(1, 2, 0, 3))  # [P, mp, k, 2P]


def _pack_w2(w: np.ndarray) -> np.ndarray:
    # [INTER, HIDDEN] f32 -> [P, MO, KI, P] bf16, block m holding cols
    # [m*128:(m+1)*128]; row f = 128*k + p
    wk = w.reshape(KI, P, MO, P).astype(BF16)       # [k, p, m, col]
    return np.ascontiguousarray(wk.transpose(1, 2, 0, 3))   # [P, m, k, col]


def _contig_cover(counts, slot_sizes, n_cores):
    """Cut the expert-sorted token stream into n_cores equal contiguous
    chunks and fit each chunk's (expert, run) pieces into the slot
    structure exactly.  Returns (core_shards, core_widx, n_entries) with
    core_shards[c][s] = (expert, expert_row0, nrows) per slot, or None
    if the counts don't fit this scheme."""
    total = sum(counts)
    tc = sum(slot_sizes)
    if total != n_cores * tc:
        return None

    # per-core contiguous pieces
    core_pieces = []
    e, used = 0, 0
    for _ in range(n_cores):
        need = tc
        pieces = []
        while need > 0:
            if e >= len(counts):
                return None
            avail = counts[e] - used
            take = min(avail, need)
            if take > 0:
                pieces.append([e, used, take])
                used += take
                need -= take
            if used >= counts[e]:
                e += 1
                used = 0
        core_pieces.append(pieces)

    gran = math.gcd(*slot_sizes)
    core_shards = []
    core_widx = []
    n_entries = 1
    for pieces in core_pieces:
        if any(p[2] % gran for p in pieces):
            return None
        # exact-fit pieces (largest first) onto slots (largest first)
        order = sorted(range(len(pieces)), key=lambda i: -pieces[i][2])
        avail = sorted(range(len(slot_sizes)), key=lambda s: -slot_sizes[s])
        slot_assign = [None] * len(slot_sizes)
        for pi in order:
            e_, r0, n = pieces[pi]
            rem = n
            r = r0
            for s in list(avail):
                sz = slot_sizes[s]
                if sz <= rem:
                    slot_assign[s] = (e_, r, sz)
                    r += sz
                    rem -= sz
                    avail.remove(s)
            if rem:
                return None
        if avail:
            return None
        # entries in first-use (slot) order; slot 0 always entry 0
        entries = []
        widx = []
        for s in range(len(slot_sizes)):
            e_ = slot_assign[s][0]
            if e_ not in entries:
                entries.append(e_)
            widx.append(entries.index(e_))
        core_shards.append(slot_assign)
        core_widx.append((widx, entries))
        n_entries = max(n_entries, len(entries))
    return core_shards, core_widx, n_entries


def _mixed_cover(counts, slot_sizes):
    """Exact-cover counts by pieces {size: N_CORES per size}. Returns
    per-core shard lists [(expert, row0, nrows), ...] ordered like
    slot_sizes, or None if no exact cover exists."""
    from collections import Counter

    sizes_desc = sorted(slot_sizes, reverse=True)
    avail = Counter(slot_sizes)
    for s in avail:
        avail[s] *= N_CORES
    per_expert: list = [None] * len(counts)

    def cover(rem, max_size):
        if rem == 0:
            return []
        for s in sorted(set(avail), reverse=True):
            if s > max_size or s > rem or avail[s] == 0:
                continue
            avail[s] -= 1
            sub = cover(rem - s, s)
            if sub is not None:
                return [s] + sub
            avail[s] += 1
        return None

    # Largest counts first so big pieces go where they must.
    order = sorted(range(len(counts)), key=lambda e: -counts[e])
    for e in order:
        pieces = cover(counts[e], max(sizes_desc))
        if pieces is None:
            return None
        per_expert[e] = pieces

    # Build shard pieces and deal them out per size class.
    by_size: dict = {s: [] for s in set(slot_sizes)}
    for e in range(len(counts)):
        r = 0
        for s in sorted(per_expert[e], reverse=True):
            by_size[s].append((e, r, s))
            r += s
    # Pad classes with empty shards (possible when sum(counts) is short).
    for s, lst in by_size.items():
        want = slot_sizes.count(s) * N_CORES
        while len(lst) < want:
            lst.append((0, 0, 0))
        if len(lst) != want:
            return None

    cores = []
    for r in range(N_CORES):
        shards = []
        used = {s: 0 for s in by_size}
        for s in slot_sizes:
            shards.append(by_size[s][r * slot_sizes.count(s) + used[s]])
            used[s] += 1
        cores.append(shards)
    return cores


def _uniform_cover(counts, slot):
    shards = []
    for e in range(len(counts)):
        r = 0
        while r < counts[e]:
            n = min(slot, counts[e] - r)
            shards.append((e, r, n))
            r += n
    n_slots = max(1, math.ceil(len(shards) / N_CORES))
    while len(shards) < N_CORES * n_slots:
        shards.append((0, 0, 0))
    return [shards[r * n_slots : (r + 1) * n_slots] for r in range(N_CORES)], n_slots


def _run(
    hidden_states: np.ndarray,
    merged_gate_up_proj: np.ndarray,
    merged_down_proj: np.ndarray,
    num_tokens_per_expert: np.ndarray,
    trace: bool = False,
):
    from concourse.bass_utils import run_bass_kernel_spmd

    counts = [int(c) for c in np.asarray(num_tokens_per_expert)]
    n_experts = len(counts)
    offs = np.concatenate([[0], np.cumsum(counts)]).astype(int)
    total = int(offs[-1])

    contig = _contig_cover(counts, MIXED_SLOTS, N_CORES)
    if contig is not None:
        core_shards, core_widx, n_entries = contig
        slot_sizes = MIXED_SLOTS
    else:
        core_shards = _mixed_cover(counts, MIXED_SLOTS)
        if core_shards is not None:
            slot_sizes = MIXED_SLOTS
        else:
            core_shards, n_slots = _uniform_cover(counts, UNIFORM_SLOT)
            slot_sizes = (UNIFORM_SLOT,) * n_slots
        # generic fallback: one weight entry per slot
        n_entries = len(slot_sizes)
        core_widx = [
            (list(range(len(slot_sizes))), [e for (e, _, _) in shards])
            for shards in core_shards
        ]

    slot_off = np.concatenate([[0], np.cumsum(slot_sizes)]).astype(int)
    T = int(slot_off[-1])

    nc = _get_program(slot_sizes, n_entries, NT)

    w1_packed = {}
    w2_packed = {}
    for shards, (widx, entries) in zip(core_shards, core_widx):
        for e in entries:
            if e not in w1_packed:
                w1_packed[e] = _pack_w1(merged_gate_up_proj[e])
                w2_packed[e] = _pack_w2(merged_down_proj[e])
    zeros_w1 = None
    zeros_w2 = None
    x_bf16 = hidden_states.astype(BF16)

    in_maps = []
    for r in range(N_CORES):
        shards = core_shards[r]
        widx, entries = core_widx[r]
        x_core = np.zeros((T, HIDDEN), dtype=BF16)
        for s, (e, r0, n) in enumerate(shards):
            if n:
                x_core[slot_off[s] : slot_off[s] + n] = x_bf16[
                    offs[e] + r0 : offs[e] + r0 + n
                ]
        # [T, HIDDEN] -> [P, KH, T] with column h = 128*k + p
        xT_core = np.ascontiguousarray(
            x_core.T.reshape(KH, P, T).transpose(1, 0, 2)
        )
        w1_list = [w1_packed[e] for e in entries]
        w2_list = [w2_packed[e] for e in entries]
        while len(w1_list) < n_entries:
            if zeros_w1 is None:
                zeros_w1 = np.zeros_like(next(iter(w1_packed.values())))
                zeros_w2 = np.zeros_like(next(iter(w2_packed.values())))
            w1_list.append(zeros_w1)
            w2_list.append(zeros_w2)
        in_maps.append(
            {
                "xT": xT_core,
                "w1": np.ascontiguousarray(np.stack(w1_list, axis=1)),
                "w2": np.ascontiguousarray(np.stack(w2_list, axis=1)),
                "widx": np.asarray([widx], dtype=np.int32),
            }
        )

    res = run_bass_kernel_spmd(nc, in_maps, list(range(N_CORES)), trace=trace)

    out = np.empty((total, HIDDEN), dtype=np.float32)
    for r in range(N_CORES):
        # [P, MO, T] -> [T, HIDDEN] with column o = 128*m + p
        o_core = res.results[r]["outT"].transpose(2, 1, 0).reshape(T, HIDDEN)
        for s, (e, r0, n) in enumerate(core_shards[r]):
            if n:
                out[offs[e] + r0 : offs[e] + r0 + n] = o_core[
                    slot_off[s] : slot_off[s] + n
                ]
    return out, res


def kernel(**inputs) -> np.ndarray:
    return _run(**inputs, trace=False)[0]


def run_traced(**inputs):
    return _run(**inputs, trace=True)


# revision 8
# speedup vs baseline: 1.1313x; 1.0304x over previous
"""Grouped MoE MLP (SwiGLU) kernel for Trainium2, 8 NeuronCores.

Strategy (expert-parallel, host-side routing):
  Tokens arrive pre-sorted by expert with per-expert counts.  The host
  cuts the expert-sorted token stream into 8 contiguous 2048-token
  chunks (one per core), so each core sees at most a couple of distinct
  experts.  Per core, the chunk's (expert, run) pieces are placed into a
  fixed slot structure (1024, 512, 256, 256); the per-core weight input
  holds only that core's distinct experts ([P, U, KH/KI, N] stacked on
  a U axis), and a tiny per-slot index tensor selects which weight
  entry each slot uses via dynamically-addressed DMA (bass.ds).  Every
  core runs the identical program: for each slot, a dense SwiGLU MLP of
  that slot's tokens with the indexed expert weights.  No device-side
  routing or collectives are needed.

  Layouts are transposed on the host so both GEMMs contract over the
  SBUF partition dimension with no on-chip transposes:
    GEMM1: out1^T[f, t] = sum_h W1[h, f] * x[t, h]   (h on partitions)
    SwiGLU on feature-partitioned tiles
    GEMM2: out^T[o, t]  = sum_f W2[f, o] * h[t, f]   (f on partitions)

  Weight DMAs ride the GpSimd (SWDGE) ring — the Pool engine is
  otherwise idle so weight streaming never queues behind compute
  semaphores; x / out / index DMAs ride the SP (sync) HWDGE ring.
  The device output is bf16 (accumulation stays fp32 in PSUM); the host
  casts back to fp32.
"""

import math
from contextlib import ExitStack

import ml_dtypes
import numpy as np

P = 128
HIDDEN = 2048
INTER = 1408
GU = 2 * INTER            # 2816 = gate+up columns
KH = HIDDEN // P          # 16 k-tiles for GEMM1
KI = INTER // P           # 11 k-tiles for GEMM2 / gate-up pair blocks
MO = HIDDEN // P          # 16 output feature blocks
N_CORES = 8
NT = 512                  # max tokens per chunk (matmul moving free dim)
MIXED_SLOTS = (1024, 512, 256, 256)   # per-core slot structure, 2048 rows
UNIFORM_SLOT = 768                    # fallback slot size

BF16 = ml_dtypes.bfloat16

_PROGRAM_CACHE: dict = {}


def _chunks(slot_rows: int, nt: int):
    out = []
    r = 0
    while r < slot_rows:
        c = min(nt, slot_rows - r)
        out.append((r, c))
        r += c
    return out


def _build_program(slot_sizes: tuple, n_entries: int, nt: int):
    import concourse.mybir as mybir
    import concourse.tile as tile
    from concourse import bacc, bass

    n_slots = len(slot_sizes)
    T = sum(slot_sizes)
    slot_off = np.concatenate([[0], np.cumsum(slot_sizes)]).astype(int)
    bf16 = mybir.dt.bfloat16
    f32 = mybir.dt.float32
    i32 = mybir.dt.int32

    nc = bacc.Bacc(None, target_bir_lowering=False, debug=False)
    xT = nc.dram_tensor("xT", [P, KH, T], bf16, kind="ExternalInput")
    # weights stored block-major so they stream in ~1.1MB / ~0.36MB pieces:
    #   w1[p, u, mp, k, 0:128]  = gate cols of output block mp
    #   w1[p, u, mp, k, 128:256] = up  cols of output block mp
    #   w2[p, u, m, k, 0:128]   = cols of output block m
    w1 = nc.dram_tensor(
        "w1", [P, n_entries, KI, KH, 2 * P], bf16, kind="ExternalInput"
    )
    w2 = nc.dram_tensor(
        "w2", [P, n_entries, MO, KI, P], bf16, kind="ExternalInput"
    )
    widx = nc.dram_tensor("widx", [1, n_slots], i32, kind="ExternalInput")
    outT = nc.dram_tensor("outT", [P, MO, T], bf16, kind="ExternalOutput")

    with tile.TileContext(nc) as tc, ExitStack() as ctx:
        w1_pool = ctx.enter_context(tc.tile_pool(name="w1p", bufs=6))
        w2_pool = ctx.enter_context(tc.tile_pool(name="w2p", bufs=6))
        x_pool = ctx.enter_context(tc.tile_pool(name="xp", bufs=2))
        h_pool = ctx.enter_context(tc.tile_pool(name="hp", bufs=2))
        g_pool = ctx.enter_context(tc.tile_pool(name="gp", bufs=2))
        o_pool = ctx.enter_context(tc.tile_pool(name="op", bufs=6))
        i_pool = ctx.enter_context(tc.tile_pool(name="ip", bufs=1))
        ps1 = ctx.enter_context(tc.tile_pool(name="ps1", bufs=3, space="PSUM"))
        ps2 = ctx.enter_context(tc.tile_pool(name="ps2", bufs=2, space="PSUM"))

        # Per-slot weight-entry indices.  Slot 0 uses entry 0 by host-side
        # convention (entries are numbered in first-use order), so its
        # weight DMAs need no register dependency and start immediately.
        wvals = [None] * n_slots
        if n_entries > 1:
            widx_t = i_pool.tile([1, n_slots], i32)
            nc.sync.dma_start(widx_t[:], widx[:])
            for s in range(1, n_slots):
                # skip_runtime_bounds_check: the device-side assert emitted
                # by the bounds check halts this runtime; indices are
                # host-generated and always within [0, n_entries).
                wvals[s] = nc.values_load(
                    widx_t[0:1, s : s + 1],
                    min_val=0,
                    max_val=n_entries - 1,
                    skip_runtime_bounds_check=True,
                )

        for s in range(n_slots):
            sz = slot_sizes[s]
            chunk_list = _chunks(sz, nt)
            t_base = int(slot_off[s])
            # whole-slot x resident in SBUF; first slot split per k-tile so
            # the first GEMM1 matmul only waits for ~256KB of x + one block
            xt = x_pool.tile([P, KH, sz], bf16, tag="xt")
            if s == 0:
                for k in range(KH):
                    nc.sync.dma_start(
                        xt[:, k : k + 1, :], xT[:, k : k + 1, t_base : t_base + sz]
                    )
            else:
                nc.sync.dma_start(xt[:], xT[:, :, t_base : t_base + sz])
            ht = h_pool.tile([P, KI, sz], bf16, tag="ht")
            for mp in range(KI):
                w1b = w1_pool.tile([P, 1, KH, 2 * P], bf16, tag="w1b")
                if wvals[s] is None:
                    nc.gpsimd.dma_start(w1b[:], w1[:, 0:1, mp, :, :])
                else:
                    nc.gpsimd.dma_start(
                        w1b[:], w1[:, bass.ds(wvals[s], 1), mp, :, :]
                    )
                for c_off, c_n in chunk_list:
                    pg = ps1.tile([P, c_n], f32, tag="pg")
                    pu = ps1.tile([P, c_n], f32, tag="pu")
                    for k in range(KH):
                        nc.tensor.matmul(
                            pg[:],
                            w1b[:, 0, k, 0:P],
                            xt[:, k, c_off : c_off + c_n],
                            start=(k == 0),
                            stop=(k == KH - 1),
                        )
                    for k in range(KH):
                        nc.tensor.matmul(
                            pu[:],
                            w1b[:, 0, k, P : 2 * P],
                            xt[:, k, c_off : c_off + c_n],
                            start=(k == 0),
                            stop=(k == KH - 1),
                        )
                    gt = g_pool.tile([P, c_n], bf16, tag="gt")
                    nc.scalar.activation(
                        gt[:], pg[:], mybir.ActivationFunctionType.Silu
                    )
                    nc.vector.tensor_mul(ht[:, mp, c_off : c_off + c_n], gt[:], pu[:])
            for m in range(MO):
                w2b = w2_pool.tile([P, 1, KI, P], bf16, tag="w2b")
                if wvals[s] is None:
                    nc.gpsimd.dma_start(w2b[:], w2[:, 0:1, m, :, :])
                else:
                    nc.gpsimd.dma_start(
                        w2b[:], w2[:, bass.ds(wvals[s], 1), m, :, :]
                    )
                for c_off, c_n in chunk_list:
                    po = ps2.tile([P, c_n], f32, tag="po")
                    for k in range(KI):
                        nc.tensor.matmul(
                            po[:],
                            w2b[:, 0, k, :],
                            ht[:, k, c_off : c_off + c_n],
                            start=(k == 0),
                            stop=(k == KI - 1),
                        )
                    om = o_pool.tile([P, c_n], bf16, tag="om")
                    nc.vector.tensor_copy(om[:], po[:])
                    nc.sync.dma_start(
                        outT[:, m, t_base + c_off : t_base + c_off + c_n], om[:]
                    )
    nc.compile()
    return nc


def _get_program(slot_sizes: tuple, n_entries: int, nt: int):
    key = (tuple(slot_sizes), n_entries, nt)
    if key not in _PROGRAM_CACHE:
        _PROGRAM_CACHE[key] = _build_program(tuple(slot_sizes), n_entries, nt)
    return _PROGRAM_CACHE[key]


def _pack_w1(w: np.ndarray) -> np.ndarray:
    # [HIDDEN, GU] f32 -> [P, KI, KH, 2P] bf16, block mp holding gate cols
    # [mp*128:(mp+1)*128] then up cols [(KI+mp)*128:(KI+mp+1)*128];
    # row h = 128*k + p
    wk = w.reshape(KH, P, GU).astype(BF16)          # [k, p, col]
    gate = wk[:, :, : KI * P].reshape(KH, P, KI, P)
    up = wk[:, :, KI * P :].reshape(KH, P, KI, P)
    blk = np.concatenate([gate, up], axis=3)        # [k, p, mp, 2P]
    return np.ascontiguousarray(blk.transpose(1, 2, 0, 3))  # [P, mp, k, 2P]


def _pack_w2(w: np.ndarray) -> np.ndarray:
    # [INTER, HIDDEN] f32 -> [P, MO, KI, P] bf16, block m holding cols
    # [m*128:(m+1)*128]; row f = 128*k + p
    wk = w.reshape(KI, P, MO, P).astype(BF16)       # [k, p, m, col]
    return np.ascontiguousarray(wk.transpose(1, 2, 0, 3))   # [P, m, k, col]


def _contig_cover(counts, slot_sizes, n_cores):
    """Cut the expert-sorted token stream into n_cores equal contiguous
    chunks and fit each chunk's (expert, run) pieces into the slot
    structure exactly.  Returns (core_shards, core_widx, n_entries) with
    core_shards[c][s] = (expert, expert_row0, nrows) per slot, or None
    if the counts don't fit this scheme."""
    total = sum(counts)
    tc = sum(slot_sizes)
    if total != n_cores * tc:
        return None

    # per-core contiguous pieces
    core_pieces = []
    e, used = 0, 0
    for _ in range(n_cores):
        need = tc
        pieces = []
        while need > 0:
            if e >= len(counts):
                return None
            avail = counts[e] - used
            take = min(avail, need)
            if take > 0:
                pieces.append([e, used, take])
                used += take
                need -= take
            if used >= counts[e]:
                e += 1
                used = 0
        core_pieces.append(pieces)

    gran = math.gcd(*slot_sizes)
    core_shards = []
    core_widx = []
    n_entries = 1
    for pieces in core_pieces:
        if any(p[2] % gran for p in pieces):
            return None
        # exact-fit pieces (largest first) onto slots (largest first)
        order = sorted(range(len(pieces)), key=lambda i: -pieces[i][2])
        avail = sorted(range(len(slot_sizes)), key=lambda s: -slot_sizes[s])
        slot_assign = [None] * len(slot_sizes)
        for pi in order:
            e_, r0, n = pieces[pi]
            rem = n
            r = r0
            for s in list(avail):
                sz = slot_sizes[s]
                if sz <= rem:
                    slot_assign[s] = (e_, r, sz)
                    r += sz
                    rem -= sz
                    avail.remove(s)
            if rem:
                return None
        if avail:
            return None
        # entries in first-use (slot) order; slot 0 always entry 0
        entries = []
        widx = []
        for s in range(len(slot_sizes)):
            e_ = slot_assign[s][0]
            if e_ not in entries:
                entries.append(e_)
            widx.append(entries.index(e_))
        core_shards.append(slot_assign)
        core_widx.append((widx, entries))
        n_entries = max(n_entries, len(entries))
    return core_shards, core_widx, n_entries


def _mixed_cover(counts, slot_sizes):
    """Exact-cover counts by pieces {size: N_CORES per size}. Returns
    per-core shard lists [(expert, row0, nrows), ...] ordered like
    slot_sizes, or None if no exact cover exists."""
    from collections import Counter

    sizes_desc = sorted(slot_sizes, reverse=True)
    avail = Counter(slot_sizes)
    for s in avail:
        avail[s] *= N_CORES
    per_expert: list = [None] * len(counts)

    def cover(rem, max_size):
        if rem == 0:
            return []
        for s in sorted(set(avail), reverse=True):
            if s > max_size or s > rem or avail[s] == 0:
                continue
            avail[s] -= 1
            sub = cover(rem - s, s)
            if sub is not None:
                return [s] + sub
            avail[s] += 1
        return None

    # Largest counts first so big pieces go where they must.
    order = sorted(range(len(counts)), key=lambda e: -counts[e])
    for e in order:
        pieces = cover(counts[e], max(sizes_desc))
        if pieces is None:
            return None
        per_expert[e] = pieces

    # Build shard pieces and deal them out per size class.
    by_size: dict = {s: [] for s in set(slot_sizes)}
    for e in range(len(counts)):
        r = 0
        for s in sorted(per_expert[e], reverse=True):
            by_size[s].append((e, r, s))
            r += s
    # Pad classes with empty shards (possible when sum(counts) is short).
    for s, lst in by_size.items():
        want = slot_sizes.count(s) * N_CORES
        while len(lst) < want:
            lst.append((0, 0, 0))
        if len(lst) != want:
            return None

    cores = []
    for r in range(N_CORES):
        shards = []
        used = {s: 0 for s in by_size}
        for s in slot_sizes:
            shards.append(by_size[s][r * slot_sizes.count(s) + used[s]])
            used[s] += 1
        cores.append(shards)
    return cores


def _uniform_cover(counts, slot):
    shards = []
    for e in range(len(counts)):
        r = 0
        while r < counts[e]:
            n = min(slot, counts[e] - r)
            shards.append((e, r, n))
            r += n
    n_slots = max(1, math.ceil(len(shards) / N_CORES))
    while len(shards) < N_CORES * n_slots:
        shards.append((0, 0, 0))
    return [shards[r * n_slots : (r + 1) * n_slots] for r in range(N_CORES)], n_slots


def _run(
    hidden_states: np.ndarray,
    merged_gate_up_proj: np.ndarray,
    merged_down_proj: np.ndarray,
    num_tokens_per_expert: np.ndarray,
    trace: bool = False,
):
    from concourse.bass_utils import run_bass_kernel_spmd

    counts = [int(c) for c in np.asarray(num_tokens_per_expert)]
    n_experts = len(counts)
    offs = np.concatenate([[0], np.cumsum(counts)]).astype(int)
    total = int(offs[-1])

    contig = _contig_cover(counts, MIXED_SLOTS, N_CORES)
    if contig is not None:
        core_shards, core_widx, n_entries = contig
        slot_sizes = MIXED_SLOTS
    else:
        core_shards = _mixed_cover(counts, MIXED_SLOTS)
        if core_shards is not None:
            slot_sizes = MIXED_SLOTS
        else:
            core_shards, n_slots = _uniform_cover(counts, UNIFORM_SLOT)
            slot_sizes = (UNIFORM_SLOT,) * n_slots
        # generic fallback: one weight entry per slot
        n_entries = len(slot_sizes)
        core_widx = [
            (list(range(len(slot_sizes))), [e for (e, _, _) in shards])
            for shards in core_shards
        ]

    slot_off = np.concatenate([[0], np.cumsum(slot_sizes)]).astype(int)
    T = int(slot_off[-1])

    nc = _get_program(slot_sizes, n_entries, NT)

    w1_packed = {}
    w2_packed = {}
    for shards, (widx, entries) in zip(core_shards, core_widx):
        for e in entries:
            if e not in w1_packed:
                w1_packed[e] = _pack_w1(merged_gate_up_proj[e])
                w2_packed[e] = _pack_w2(merged_down_proj[e])
    zeros_w1 = None
    zeros_w2 = None
    x_bf16 = hidden_states.astype(BF16)

    in_maps = []
    for r in range(N_CORES):
        shards = core_shards[r]
        widx, entries = core_widx[r]
        x_core = np.zeros((T, HIDDEN), dtype=BF16)
        for s, (e, r0, n) in enumerate(shards):
            if n:
                x_core[slot_off[s] : slot_off[s] + n] = x_bf16[
                    offs[e] + r0 : offs[e] + r0 + n
                ]
        # [T, HIDDEN] -> [P, KH, T] with column h = 128*k + p
        xT_core = np.ascontiguousarray(
            x_core.T.reshape(KH, P, T).transpose(1, 0, 2)
        )
        w1_list = [w1_packed[e] for e in entries]
        w2_list = [w2_packed[e] for e in entries]
        while len(w1_list) < n_entries:
            if zeros_w1 is None:
                zeros_w1 = np.zeros_like(next(iter(w1_packed.values())))
                zeros_w2 = np.zeros_like(next(iter(w2_packed.values())))
            w1_list.append(zeros_w1)
            w2_list.append(zeros_w2)
        in_maps.append(
            {
                "xT": xT_core,
                "w1": np.ascontiguousarray(np.stack(w1_list, axis=1)),
                "w2": np.ascontiguousarray(np.stack(w2_list, axis=1)),
                "widx": np.asarray([widx], dtype=np.int32),
            }
        )

    res = run_bass_kernel_spmd(nc, in_maps, list(range(N_CORES)), trace=trace)

    out = np.empty((total, HIDDEN), dtype=np.float32)
    for r in range(N_CORES):
        # [P, MO, T] -> [T, HIDDEN] with column o = 128*m + p
        o_core = res.results[r]["outT"].transpose(2, 1, 0).reshape(T, HIDDEN)
        for s, (e, r0, n) in enumerate(core_shards[r]):
            if n:
                out[offs[e] + r0 : offs[e] + r0 + n] = o_core[
                    slot_off[s] : slot_off[s] + n
                ]
    return out, res


def kernel(**inputs) -> np.ndarray:
    return _run(**inputs, trace=False)[0]


def run_traced(**inputs):
    return _run(**inputs, trace=True)
